# revision 16
# baseline (speedup 1.0000x reference)
"""SimCLR contrastive loss (NT-Xent) on 8 Trainium2 NeuronCores.

Reference:
    z  = concat(z_i, z_j)                     # [N, D], N = 8192, D = 256
    zn = z / max(||z||_row, eps)
    sim = zn @ zn.T / TEMP                    # TEMP = 0.5
    lse = logsumexp(sim with -inf diagonal, axis=1)
    pos[r] = sim[r, (r + B) mod N]
    loss = sum(lse - pos) / N

Distribution: data-parallel over rows.  Core c owns rows [1024c, 1024c+1024);
the host ships each core the *row-rotated* embeddings so one SPMD program
serves all cores (own rows are always local columns [0, 1024), the positive
window for row-tile m is local columns [4096+128m, 4096+128m+128), and the
diagonal is folded out by subtracting the constant e^2 from each row sum).

Per-core pipeline (v2: fp8 DoubleRow matmul + split exp):
  Host ships two bf16 layouts of the rotated z (pure layout work):
    ztp  [128, 2, 8192]: ztp[p, j, c] = z_loc[c, p + 128j]   (transposed, the
         two 128-deep contraction tiles side by side for DoubleRow packing)
    zrow [128, 16384]:   zrow[p, 64t + ...] = z_loc[128t + p, :]  (row-major,
         for single-pass row-norm computation)
  1. Norms: 64x DVE tensor_tensor_reduce (z*z with accum) -> ss [128, 64]
     in the transposed layout directly; Quake-seed Newton rsqrt on DVE
     (ACT Rsqrt LUT is banned for accuracy); inv -> DRAM (bf16) -> stride-0
     broadcast DMA -> bc [128, 8192]; znb = ztp * bc cast to fp8e4.
  2. Main loop (4 sweeps x 8 row tiles): each [128, 2048] PSUM tile filled by
     4 DoubleRow fp8 matmuls (256-deep contraction, 2 elem/cycle); consumed by
     either ScalarE (exp + fused row-sum accumulate, scale=2 folds 1/TEMP) or
     VectorE (Schraudolph fast-exp: (x*A + B) -> int16 bit pattern == bf16
     exp approximation, then a 4x-mode tensor_scalar with accum_out row-sums
     it; the magic constant zeroes the mean sawtooth bias).
  3. lse = ln(rowsum - e^2); out = lse - pos; host sums in fp64 / N.
"""

import os
import sys

import numpy as np

B = 4096
D = 256
N = 2 * B
NCORES = 8
RPC = N // NCORES  # rows per core

_CANDIDATE_PATHS = ("/opt/trn_rl_repo", "/root/.axon_site/_ro/trn_rl_repo")


def _ensure_import_path():
    try:
        import concourse.bass  # noqa: F401
        return
    except ImportError:
        pass
    for p in _CANDIDATE_PATHS:
        if os.path.isdir(p) and p not in sys.path:
            sys.path.insert(0, p)
    import concourse.bass  # noqa: F401


# Schraudolph fast-exp constants for exp(2*x) via bf16 bit pattern:
#   bits = round(x * 2*128*log2(e) + (127*128 - 128*log2(Eg)))
# where Eg = E_f[(1+f)/2^f] = 1.0406984 zeroes the mean sawtooth bias.
SCH_A = 369.32993046757464          # 2 * 128 * log2(e)
SCH_B = 16248.633                   # 16256 - 128*log2(1.0406984)
EXP2 = float(np.exp(2.0))           # exp(sim[i,i] * 2) subtracted per row

USE_FP8 = True


def build_program(n=N, d=D, rpc=RPC):
    _ensure_import_path()
    from contextlib import ExitStack

    import concourse.bacc as bacc
    import concourse.tile as tile
    from concourse import mybir

    f32 = mybir.dt.float32
    bf16 = mybir.dt.bfloat16
    fp8 = mybir.dt.float8e4
    i16 = mybir.dt.int16
    i32 = mybir.dt.int32
    FT = mybir.ActivationFunctionType
    OP = mybir.AluOpType
    DR = mybir.MatmulPerfMode.DoubleRow

    P = 128
    CH = 512                    # one fp32 PSUM bank
    GW = 2048                   # sweep/group width (4 banks)
    nsw = n // GW               # 4 sweeps
    mt = rpc // P               # 8 row tiles
    tpg = GW // P               # norm t-chunks per group (16)
    EG = GW // CH               # 512-chunks per sweep (4)

    # tiles whose exp+rowsum runs on VectorE (the rest go to ScalarE)
    DVE_TILES = {(1, 3), (1, 6), (2, 2), (2, 5), (3, 1), (3, 3), (3, 5), (3, 7)}

    nc = bacc.Bacc("TRN2", target_bir_lowering=False, debug=False)
    ztp_d = nc.dram_tensor("ztp", [P, 2, n], bf16, kind="ExternalInput").ap()
    zrow_d = nc.dram_tensor("zrow", [P, (n // P) * d], bf16, kind="ExternalInput").ap()
    id_d = nc.dram_tensor("ident", [P, P], f32, kind="ExternalInput").ap()
    out_d = nc.dram_tensor("out", [P, mt], f32, kind="ExternalOutput").ap()
    invd = nc.dram_tensor("invd", [1, n], bf16).ap()   # scratch: 1/norm

    with tile.TileContext(nc) as tc, ExitStack() as ctx:
        big = ctx.enter_context(tc.tile_pool(name="big", bufs=1))
        small = ctx.enter_context(tc.tile_pool(name="small", bufs=2))
        i16p = ctx.enter_context(tc.tile_pool(name="i16p", bufs=2))
        stat = ctx.enter_context(tc.tile_pool(name="stat", bufs=1))
        mps = ctx.enter_context(tc.tile_pool(name="mps", bufs=2, space="PSUM"))

        zt = big.tile([P, 2, n], bf16, tag="zt")
        zrow = big.tile([P, (n // P) * d], bf16, tag="zrow")
        mm_dt = fp8 if USE_FP8 else bf16
        znb = big.tile([P, 2, n], mm_dt, tag="znb")
        zn16 = big.tile([P, 2, n], bf16, tag="zn16")
        jnk = big.tile([P, GW], bf16, tag="jnk")       # DVE exp-sum junk out
        sqb = big.tile([P, tpg * d], bf16, tag="sqb")  # squares scratch
        tjnk = stat.tile([P, d], bf16, tag="tjnk")     # ttr junk out
        ident_sb = stat.tile([P, P], f32, tag="ident")
        ssg = [stat.tile([P, tpg], f32, tag=f"ss{g}", name=f"ss{g}") for g in range(nsw)]
        invg = [stat.tile([P, tpg], bf16, tag=f"inv{g}", name=f"inv{g}") for g in range(nsw)]
        irow = [stat.tile([1, GW], bf16, tag=f"ir{g}", name=f"ir{g}") for g in range(nsw)]
        ones1 = stat.tile([1, P], bf16, tag="ones1")
        bcps_t = [None] * nsw
        out_sb = stat.tile([P, mt], f32, tag="out_sb")
        partsA = stat.tile([P, mt, nsw], f32, tag="partsA")
        partsB = stat.tile([P, mt, nsw], f32, tag="partsB")
        poss = stat.tile([P, mt], f32, tag="poss")

        nc.sync.dma_start(out=ident_sb, in_=id_d)
        nc.vector.memset(ones1, 1.0)
        nc.vector.memset(partsA, 0.0)
        nc.vector.memset(partsB, 0.0)

        # ---- input streams: interleave the two layouts group by group so
        # group g's norm inputs and matmul inputs both arrive early; the SP
        # HWDGE FIFO carries only these (no dependent DMAs -> no head-of-line
        # stalls) and drains at HBM line rate.
        for g in range(nsw):
            zr = slice(g * P * tpg * 2, (g + 1) * P * tpg * 2)  # 4096 cols
            nc.sync.dma_start(out=zrow[:, zr], in_=zrow_d[:, zr])
            G = slice(GW * g, GW * (g + 1))
            nc.sync.dma_start(out=zt[:, :, G], in_=ztp_d[:, :, G])

        def norms_group(g):
            """ss for rows [2048g, 2048(g+1)): square (2x bf16 TT) then one
            3D tensor_reduce over the innermost 256-wide row chunks."""
            zc = slice(tpg * g * d, tpg * (g + 1) * d)
            nc.vector.tensor_mul(sqb, zrow[:, zc], zrow[:, zc])
            nc.vector.tensor_reduce(
                out=ssg[g],
                in_=sqb.rearrange("p (t d) -> p t d", d=d),
                axis=mybir.AxisListType.X,
                op=OP.add,
            )

        def norms_group_act(g):
            """Same, on the (idle during prologue) Scalar engine via
            Square with fused accumulate, one 256-chunk at a time."""
            for t in range(tpg):
                zc = slice((tpg * g + t) * d, (tpg * g + t + 1) * d)
                nc.scalar.activation(
                    out=tjnk, in_=zrow[:, zc], func=FT.Square,
                    accum_out=ssg[g][:, t : t + 1],
                )

        def newton_group(g):
            """inv = 1/sqrt(ss) for group g: Quake seed + 2 Newton steps."""
            sg = ssg[g]
            ii = small.tile([P, tpg], i32, tag="ii")
            nc.vector.tensor_scalar(
                out=ii, in0=sg.bitcast(i32), scalar1=1, scalar2=None,
                op0=OP.arith_shift_right,
            )
            nc.vector.tensor_scalar(
                out=ii, in0=ii, scalar1=-1, scalar2=None, op0=OP.bitwise_xor
            )
            nc.vector.tensor_scalar(
                out=ii, in0=ii, scalar1=0x5F3759DF + 1, scalar2=None, op0=OP.add
            )
            y = ii.bitcast(f32)
            t_ = small.tile([P, tpg], f32, tag="t_")
            for _ in range(2):
                nc.vector.tensor_mul(t_, y, y)
                nc.vector.tensor_mul(t_, t_, sg)
                nc.vector.tensor_scalar(
                    out=t_, in0=t_, scalar1=-0.5, scalar2=1.5,
                    op0=OP.mult, op1=OP.add,
                )
                nc.vector.tensor_mul(y, y, t_)
            nc.vector.tensor_copy(out=invg[g], in_=y)

        def bcast_group(g):
            """inv -> DRAM in column order -> stride-0 broadcast to bc.
            Issued from the ACT HWDGE ring: ACT is idle during the prologue,
            and the gpsimd SWDGE ring gets serialized against DVE perf-mode
            instructions (observed 30us delays)."""
            G = slice(GW * g, GW * (g + 1))
            nc.sync.dma_start(
                out=invd[0, G].rearrange("(t p) -> p t", p=P),
                in_=invg[g],
            )
            nc.sync.dma_start(out=irow[g], in_=invd[:, G])
            bcps = mps.tile([P, GW], f32, tag="ps", name=f"bcps{g}")
            for c in range(EG):
                nc.tensor.matmul(
                    bcps[:, CH * c : CH * (c + 1)],
                    ones1,
                    irow[g][0:1, CH * c : CH * (c + 1)],
                    start=True,
                    stop=True,
                )
            bcps_t[g] = bcps

        def normalize_group(g):
            G = slice(GW * g, GW * (g + 1))
            for j in range(2):
                nc.vector.tensor_mul(zn16[:, j, G], zt[:, j, G], bcps_t[g])
            nc.vector.tensor_copy(out=znb[:, :, G], in_=zn16[:, :, G])

        def warmup(src, k, nmm):
            """Dummy matmuls on already-loaded data keep the PE HAM warm
            (any >3.4us idle gap drops the PE clock 2.4 -> 1.2 GHz)."""
            wps = mps.tile([P, GW], f32, tag="ps", name=f"wu{k}")
            for i in range(nmm):
                nc.tensor.matmul(
                    wps[:, 0:CH], src[:, 0:P], src[:, 0:CH],
                    start=True, stop=True,
                )

        # Prologue pipeline: DVE order chosen so group 0's chain finishes
        # fastest while later groups' work fills the DMA-wait gaps; PE
        # warmup matmuls are spaced by DMA arrivals to hold the clock high.
        norms_group(0)
        newton_group(0)
        norms_group_act(2)
        norms_group_act(3)
        warmup(zrow, 0, 2)
        warmup(zt[:, 0, :], 1, 2)
        bcast_group(0)
        norms_group(1)
        newton_group(1)
        bcast_group(1)
        normalize_group(0)
        warmup(zrow[:, tpg * d :], 2, 2)
        warmup(zt[:, 1, GW : 2 * GW], 3, 6)
        newton_group(2)
        bcast_group(2)
        normalize_group(1)
        newton_group(3)
        bcast_group(3)
        normalize_group(2)
        normalize_group(3)

        def main_tile(s, m):
            ps = mps.tile([P, GW], f32, tag="ps", name=f"ps_{s}_{m}")
            for c in range(EG):
                cols = slice(GW * s + CH * c, GW * s + CH * (c + 1))
                if USE_FP8:
                    nc.tensor.matmul(
                        ps[:, CH * c : CH * (c + 1)],
                        znb[:, :, P * m : P * (m + 1)],
                        znb[:, :, cols],
                        start=True,
                        stop=True,
                        perf_mode=DR,
                    )
                else:
                    for j in range(2):
                        nc.tensor.matmul(
                            ps[:, CH * c : CH * (c + 1)],
                            znb[:, j, P * m : P * (m + 1)],
                            znb[:, j, cols],
                            start=(j == 0),
                            stop=(j == 1),
                        )
            w0 = n // 2 + P * m
            if w0 // GW == s:  # positive-pair window lives in this sweep
                off = w0 % GW
                junk = small.tile([P, P], f32, tag="pjunk")
                nc.vector.scalar_tensor_tensor(
                    out=junk,
                    in0=ps[:, off : off + P],
                    scalar=2.0,
                    in1=ident_sb,
                    op0=OP.mult,
                    op1=OP.mult,
                    accum_out=poss[:, m : m + 1],
                )
            if (s, m) in DVE_TILES:
                ib = i16p.tile([P, GW], i16, tag="ib", name=f"ib_{s}_{m}")
                nc.vector.tensor_scalar(
                    out=ib, in0=ps, scalar1=SCH_A, scalar2=SCH_B,
                    op0=OP.mult, op1=OP.add,
                )
                nc.vector.tensor_scalar(
                    out=jnk, in0=ib.bitcast(bf16), scalar1=1.0, scalar2=None,
                    op0=OP.mult, op1=OP.add,
                    accum_out=partsB[:, m, s : s + 1],
                )
            else:
                nc.scalar.activation(
                    out=ps,
                    in_=ps,
                    func=FT.Exp,
                    scale=2.0,
                    accum_out=partsA[:, m, s : s + 1],
                )

        for s in range(nsw):
            for m in range(mt):
                main_tile(s, m)

        # ---- Per-row finalization ----
        for m in range(mt):
            SA = small.tile([P, 1], f32, tag="SA")
            nc.vector.tensor_reduce(
                out=SA, in_=partsA[:, m, :], axis=mybir.AxisListType.X, op=OP.add
            )
            SB = small.tile([P, 1], f32, tag="SB")
            nc.vector.tensor_reduce(
                out=SB, in_=partsB[:, m, :], axis=mybir.AxisListType.X, op=OP.add
            )
            nc.vector.tensor_add(SA, SA, SB)
            nc.vector.tensor_scalar_add(SA, SA, -EXP2)
            lse = small.tile([P, 1], f32, tag="lse")
            nc.scalar.activation(out=lse, in_=SA, func=FT.Ln)
            nc.vector.tensor_tensor(
                out=out_sb[:, m : m + 1], in0=lse, in1=poss[:, m : m + 1],
                op=OP.subtract,
            )
        nc.sync.dma_start(out=out_d, in_=out_sb)

    nc.compile()
    return nc


def make_in_maps(z_i, z_j, n=N, d=D, rpc=RPC, ncores=NCORES):
    """Host-side sharding: two rotated bf16 layouts per core (layout only)."""
    import ml_dtypes

    P = 128
    z = np.concatenate(
        [np.asarray(z_i, dtype=np.float32), np.asarray(z_j, dtype=np.float32)],
        axis=0,
    )
    ident = np.eye(P, dtype=np.float32)
    in_maps = []
    for c in range(ncores):
        z_loc = np.roll(z, -rpc * c, axis=0)              # [N, D]
        zT = z_loc.T                                      # [D, N]
        ztp = np.ascontiguousarray(
            zT.reshape(2, P, n).transpose(1, 0, 2)
        ).astype(ml_dtypes.bfloat16)                      # [128, 2, N]
        zrow = np.ascontiguousarray(
            z_loc.reshape(n // P, P, d).transpose(1, 0, 2).reshape(P, -1)
        ).astype(ml_dtypes.bfloat16)                      # [128, (N/128)*D]
        in_maps.append({"ztp": ztp, "zrow": zrow, "ident": ident})
    return in_maps


def gather_loss(results, n=N):
    total = 0.0
    for r in results:
        total += np.asarray(r["out"], dtype=np.float64).sum()
    return np.float32(total / n)


_PROGRAM_CACHE = {}


def kernel(z_i, z_j):
    _ensure_import_path()
    from concourse.bass_utils import run_bass_kernel_spmd

    key = (N, D, RPC)
    if key not in _PROGRAM_CACHE:
        _PROGRAM_CACHE[key] = build_program()
    nc = _PROGRAM_CACHE[key]
    in_maps = make_in_maps(z_i, z_j)
    results = run_bass_kernel_spmd(nc, in_maps, list(range(NCORES))).results
    return gather_loss(results)


if __name__ == "__main__":
    rng = np.random.default_rng(0)
    z_i = rng.standard_normal((B, D), dtype=np.float32)
    z_j = rng.standard_normal((B, D), dtype=np.float32)
    loss = kernel(z_i, z_j)
    print("loss:", loss)


# revision 17
# speedup vs baseline: 1.1051x; 1.1051x over previous
"""SimCLR contrastive loss (NT-Xent) on 8 Trainium2 NeuronCores.

Reference:
    z  = concat(z_i, z_j)                     # [N, D], N = 8192, D = 256
    zn = z / max(||z||_row, eps)
    sim = zn @ zn.T / TEMP                    # TEMP = 0.5
    lse = logsumexp(sim with -inf diagonal, axis=1)
    pos[r] = sim[r, (r + B) mod N]
    loss = sum(lse - pos) / N

Distribution: data-parallel over rows.  Core c owns rows [1024c, 1024c+1024);
the host ships each core the *row-rotated* embeddings so one SPMD program
serves all cores (own rows are always local columns [0, 1024), the positive
window for row-tile m is local columns [4096+128m, 4096+128m+128), and the
diagonal is folded out by subtracting the constant e^2 from each row sum).

Per-core pipeline (v2: fp8 DoubleRow matmul + split exp):
  Host ships two bf16 layouts of the rotated z (pure layout work):
    ztp  [128, 2, 8192]: ztp[p, j, c] = z_loc[c, p + 128j]   (transposed, the
         two 128-deep contraction tiles side by side for DoubleRow packing)
    zrow [128, 16384]:   zrow[p, 64t + ...] = z_loc[128t + p, :]  (row-major,
         for single-pass row-norm computation)
  1. Norms: 64x DVE tensor_tensor_reduce (z*z with accum) -> ss [128, 64]
     in the transposed layout directly; Quake-seed Newton rsqrt on DVE
     (ACT Rsqrt LUT is banned for accuracy); inv -> DRAM (bf16) -> stride-0
     broadcast DMA -> bc [128, 8192]; znb = ztp * bc cast to fp8e4.
  2. Main loop (4 sweeps x 8 row tiles): each [128, 2048] PSUM tile filled by
     4 DoubleRow fp8 matmuls (256-deep contraction, 2 elem/cycle); consumed by
     either ScalarE (exp + fused row-sum accumulate, scale=2 folds 1/TEMP) or
     VectorE (Schraudolph fast-exp: (x*A + B) -> int16 bit pattern == bf16
     exp approximation, then a 4x-mode tensor_scalar with accum_out row-sums
     it; the magic constant zeroes the mean sawtooth bias).
  3. lse = ln(rowsum - e^2); out = lse - pos; host sums in fp64 / N.
"""

import os
import sys

import numpy as np

B = 4096
D = 256
N = 2 * B
NCORES = 8
RPC = N // NCORES  # rows per core

_CANDIDATE_PATHS = ("/opt/trn_rl_repo", "/root/.axon_site/_ro/trn_rl_repo")


def _ensure_import_path():
    try:
        import concourse.bass  # noqa: F401
        return
    except ImportError:
        pass
    for p in _CANDIDATE_PATHS:
        if os.path.isdir(p) and p not in sys.path:
            sys.path.insert(0, p)
    import concourse.bass  # noqa: F401


# Schraudolph fast-exp constants for exp(2*x) via bf16 bit pattern:
#   bits = round(x * 2*128*log2(e) + (127*128 - 128*log2(Eg)))
# where Eg = E_f[(1+f)/2^f] = 1.0406984 zeroes the mean sawtooth bias.
SCH_A = 369.32993046757464          # 2 * 128 * log2(e)
SCH_B = 16248.633                   # 16256 - 128*log2(1.0406984)
EXP2 = float(np.exp(2.0))           # exp(sim[i,i] * 2) subtracted per row

USE_FP8 = True


def build_program(n=N, d=D, rpc=RPC):
    _ensure_import_path()
    from contextlib import ExitStack

    import concourse.bacc as bacc
    import concourse.tile as tile
    from concourse import mybir

    f32 = mybir.dt.float32
    bf16 = mybir.dt.bfloat16
    fp8 = mybir.dt.float8e4
    i16 = mybir.dt.int16
    i32 = mybir.dt.int32
    FT = mybir.ActivationFunctionType
    OP = mybir.AluOpType
    DR = mybir.MatmulPerfMode.DoubleRow

    P = 128
    CH = 512                    # one fp32 PSUM bank
    GW = 2048                   # sweep/group width (4 banks)
    nsw = n // GW               # 4 sweeps
    mt = rpc // P               # 8 row tiles
    tpg = GW // P               # norm t-chunks per group (16)
    EG = GW // CH               # 512-chunks per sweep (4)

    # tiles whose exp+rowsum runs on VectorE (the rest go to ScalarE)
    DVE_TILES = {(1, 3), (1, 6), (2, 2), (2, 5), (3, 1), (3, 3), (3, 5), (3, 7)}

    nc = bacc.Bacc("TRN2", target_bir_lowering=False, debug=False)
    ztp_d = nc.dram_tensor("ztp", [P, 2, n], bf16, kind="ExternalInput").ap()
    zrow_d = nc.dram_tensor("zrow", [P, (n // P) * d], bf16, kind="ExternalInput").ap()
    id_d = nc.dram_tensor("ident", [P, P], f32, kind="ExternalInput").ap()
    out_d = nc.dram_tensor("out", [P, mt], f32, kind="ExternalOutput").ap()
    invd = nc.dram_tensor("invd", [1, n], bf16).ap()   # scratch: 1/norm

    with tile.TileContext(nc) as tc, ExitStack() as ctx:
        big = ctx.enter_context(tc.tile_pool(name="big", bufs=1))
        small = ctx.enter_context(tc.tile_pool(name="small", bufs=2))
        i16p = ctx.enter_context(tc.tile_pool(name="i16p", bufs=2))
        stat = ctx.enter_context(tc.tile_pool(name="stat", bufs=1))
        mps = ctx.enter_context(tc.tile_pool(name="mps", bufs=2, space="PSUM"))

        zt = big.tile([P, 2, n], bf16, tag="zt")
        zrow = big.tile([P, (n // P) * d], bf16, tag="zrow")
        mm_dt = fp8 if USE_FP8 else bf16
        znb = big.tile([P, 2, n], mm_dt, tag="znb")
        zn16 = big.tile([P, 2, n], bf16, tag="zn16")
        jnk = big.tile([P, GW], bf16, tag="jnk")       # DVE exp-sum junk out
        sqb = big.tile([P, tpg * d], bf16, tag="sqb")  # squares scratch
        tjnk = stat.tile([P, d], bf16, tag="tjnk")     # ttr junk out
        ident_sb = stat.tile([P, P], f32, tag="ident")
        ssg = [stat.tile([P, tpg], f32, tag=f"ss{g}", name=f"ss{g}") for g in range(nsw)]
        invg = [stat.tile([P, tpg], bf16, tag=f"inv{g}", name=f"inv{g}") for g in range(nsw)]
        irow = [stat.tile([1, GW], bf16, tag=f"ir{g}", name=f"ir{g}") for g in range(nsw)]
        ones1 = stat.tile([1, P], bf16, tag="ones1")
        bcps_t = [None] * nsw
        out_sb = stat.tile([P, mt], f32, tag="out_sb")
        partsA = stat.tile([P, mt, nsw], f32, tag="partsA")
        partsB = stat.tile([P, mt, nsw], f32, tag="partsB")
        poss = stat.tile([P, mt], f32, tag="poss")

        nc.sync.dma_start(out=ident_sb, in_=id_d)
        nc.vector.memset(ones1, 1.0)
        nc.vector.memset(partsA, 0.0)
        nc.vector.memset(partsB, 0.0)

        # ---- input streams: interleave the two layouts group by group so
        # group g's norm inputs and matmul inputs both arrive early; the SP
        # HWDGE FIFO carries only these (no dependent DMAs -> no head-of-line
        # stalls) and drains at HBM line rate.
        for g in range(nsw):
            zr = slice(g * P * tpg * 2, (g + 1) * P * tpg * 2)  # 4096 cols
            nc.sync.dma_start(out=zrow[:, zr], in_=zrow_d[:, zr])
            G = slice(GW * g, GW * (g + 1))
            nc.sync.dma_start(out=zt[:, :, G], in_=ztp_d[:, :, G])

        def norms_group(g):
            """ss for rows [2048g, 2048(g+1)): square (2x bf16 TT) then one
            3D tensor_reduce over the innermost 256-wide row chunks."""
            zc = slice(tpg * g * d, tpg * (g + 1) * d)
            nc.vector.tensor_mul(sqb, zrow[:, zc], zrow[:, zc])
            nc.vector.tensor_reduce(
                out=ssg[g],
                in_=sqb.rearrange("p (t d) -> p t d", d=d),
                axis=mybir.AxisListType.X,
                op=OP.add,
            )

        def norms_group_act(g):
            """Same, on the (idle during prologue) Scalar engine via
            Square with fused accumulate, one 256-chunk at a time."""
            for t in range(tpg):
                zc = slice((tpg * g + t) * d, (tpg * g + t + 1) * d)
                nc.scalar.activation(
                    out=tjnk, in_=zrow[:, zc], func=FT.Square,
                    accum_out=ssg[g][:, t : t + 1],
                )

        def newton_group(g):
            """inv = 1/sqrt(ss) for group g: Quake seed + 2 Newton steps."""
            sg = ssg[g]
            ii = small.tile([P, tpg], i32, tag="ii")
            nc.vector.tensor_scalar(
                out=ii, in0=sg.bitcast(i32), scalar1=1, scalar2=None,
                op0=OP.arith_shift_right,
            )
            nc.vector.tensor_scalar(
                out=ii, in0=ii, scalar1=-1, scalar2=None, op0=OP.bitwise_xor
            )
            nc.vector.tensor_scalar(
                out=ii, in0=ii, scalar1=0x5F3759DF + 1, scalar2=None, op0=OP.add
            )
            y = ii.bitcast(f32)
            t_ = small.tile([P, tpg], f32, tag="t_")
            for _ in range(2):
                nc.vector.tensor_mul(t_, y, y)
                nc.vector.tensor_mul(t_, t_, sg)
                nc.vector.tensor_scalar(
                    out=t_, in0=t_, scalar1=-0.5, scalar2=1.5,
                    op0=OP.mult, op1=OP.add,
                )
                nc.vector.tensor_mul(y, y, t_)
            nc.vector.tensor_copy(out=invg[g], in_=y)

        def bcast_group(g):
            """inv -> DRAM in column order -> stride-0 broadcast to bc.
            Issued from the ACT HWDGE ring: ACT is idle during the prologue,
            and the gpsimd SWDGE ring gets serialized against DVE perf-mode
            instructions (observed 30us delays)."""
            G = slice(GW * g, GW * (g + 1))
            nc.gpsimd.dma_start(
                out=invd[0, G].rearrange("(t p) -> p t", p=P),
                in_=invg[g],
            )
            nc.gpsimd.dma_start(out=irow[g], in_=invd[:, G])


        def bcmm_group(g):
            bcps = mps.tile([P, GW], f32, tag="ps", name=f"bcps{g}")
            for c in range(EG):
                nc.tensor.matmul(
                    bcps[:, CH * c : CH * (c + 1)],
                    ones1,
                    irow[g][0:1, CH * c : CH * (c + 1)],
                    start=True,
                    stop=True,
                )
            bcps_t[g] = bcps

        def normalize_group(g):
            G = slice(GW * g, GW * (g + 1))
            for j in range(2):
                nc.vector.tensor_mul(zn16[:, j, G], zt[:, j, G], bcps_t[g])
            nc.vector.tensor_copy(out=znb[:, :, G], in_=zn16[:, :, G])

        def warmup(src, k, nmm):
            """Dummy matmuls on already-loaded data keep the PE HAM warm
            (any >3.4us idle gap drops the PE clock 2.4 -> 1.2 GHz)."""
            wps = mps.tile([P, GW], f32, tag="ps", name=f"wu{k}")
            for i in range(nmm):
                nc.tensor.matmul(
                    wps[:, 0:CH], src[:, 0:P], src[:, 0:CH],
                    start=True, stop=True,
                )

        # Prologue pipeline: DVE order chosen so group 0's chain finishes
        # fastest while later groups' work fills the DMA-wait gaps; PE
        # warmup matmuls are spaced by DMA arrivals to hold the clock high.
        norms_group(0)
        newton_group(0)
        norms_group_act(2)
        warmup(zrow, 0, 2)
        warmup(zt[:, 0, :], 1, 2)
        bcast_group(0)
        norms_group(1)
        newton_group(1)
        bcast_group(1)
        warmup(zrow[:, tpg * d :], 2, 2)
        bcmm_group(0)
        normalize_group(0)
        newton_group(2)
        bcast_group(2)

        def main_tile(s, m):
            ps = mps.tile([P, GW], f32, tag="ps", name=f"ps_{s}_{m}")
            for c in range(EG):
                cols = slice(GW * s + CH * c, GW * s + CH * (c + 1))
                if USE_FP8:
                    nc.tensor.matmul(
                        ps[:, CH * c : CH * (c + 1)],
                        znb[:, :, P * m : P * (m + 1)],
                        znb[:, :, cols],
                        start=True,
                        stop=True,
                        perf_mode=DR,
                    )
                else:
                    for j in range(2):
                        nc.tensor.matmul(
                            ps[:, CH * c : CH * (c + 1)],
                            znb[:, j, P * m : P * (m + 1)],
                            znb[:, j, cols],
                            start=(j == 0),
                            stop=(j == 1),
                        )
            w0 = n // 2 + P * m
            if w0 // GW == s:  # positive-pair window lives in this sweep
                off = w0 % GW
                junk = small.tile([P, P], f32, tag="pjunk")
                nc.vector.scalar_tensor_tensor(
                    out=junk,
                    in0=ps[:, off : off + P],
                    scalar=2.0,
                    in1=ident_sb,
                    op0=OP.mult,
                    op1=OP.mult,
                    accum_out=poss[:, m : m + 1],
                )
            if (s, m) in DVE_TILES:
                ib = i16p.tile([P, GW], i16, tag="ib", name=f"ib_{s}_{m}")
                nc.vector.tensor_scalar(
                    out=ib, in0=ps, scalar1=SCH_A, scalar2=SCH_B,
                    op0=OP.mult, op1=OP.add,
                )
                nc.vector.tensor_scalar(
                    out=jnk, in0=ib.bitcast(bf16), scalar1=1.0, scalar2=None,
                    op0=OP.mult, op1=OP.add,
                    accum_out=partsB[:, m, s : s + 1],
                )
            else:
                nc.scalar.activation(
                    out=ps,
                    in_=ps,
                    func=FT.Exp,
                    scale=2.0,
                    accum_out=partsA[:, m, s : s + 1],
                )

        tseq = [(s, m) for s in range(nsw) for m in range(mt)]
        for k, (s, m) in enumerate(tseq):
            main_tile(s, m)
            if k == 1:
                bcmm_group(1)
                normalize_group(1)
            elif k == 4:
                norms_group_act(3)
            elif k == 6:
                bcmm_group(2)
                normalize_group(2)
                newton_group(3)
                bcast_group(3)
            elif k == 12:
                bcmm_group(3)
                normalize_group(3)

        # ---- Per-row finalization ----
        for m in range(mt):
            SA = small.tile([P, 1], f32, tag="SA")
            nc.vector.tensor_reduce(
                out=SA, in_=partsA[:, m, :], axis=mybir.AxisListType.X, op=OP.add
            )
            SB = small.tile([P, 1], f32, tag="SB")
            nc.vector.tensor_reduce(
                out=SB, in_=partsB[:, m, :], axis=mybir.AxisListType.X, op=OP.add
            )
            nc.vector.tensor_add(SA, SA, SB)
            nc.vector.tensor_scalar_add(SA, SA, -EXP2)
            lse = small.tile([P, 1], f32, tag="lse")
            nc.scalar.activation(out=lse, in_=SA, func=FT.Ln)
            nc.vector.tensor_tensor(
                out=out_sb[:, m : m + 1], in0=lse, in1=poss[:, m : m + 1],
                op=OP.subtract,
            )
        nc.sync.dma_start(out=out_d, in_=out_sb)

    nc.compile()
    return nc


def make_in_maps(z_i, z_j, n=N, d=D, rpc=RPC, ncores=NCORES):
    """Host-side sharding: two rotated bf16 layouts per core (layout only)."""
    import ml_dtypes

    P = 128
    z = np.concatenate(
        [np.asarray(z_i, dtype=np.float32), np.asarray(z_j, dtype=np.float32)],
        axis=0,
    )
    ident = np.eye(P, dtype=np.float32)
    in_maps = []
    for c in range(ncores):
        z_loc = np.roll(z, -rpc * c, axis=0)              # [N, D]
        zT = z_loc.T                                      # [D, N]
        ztp = np.ascontiguousarray(
            zT.reshape(2, P, n).transpose(1, 0, 2)
        ).astype(ml_dtypes.bfloat16)                      # [128, 2, N]
        zrow = np.ascontiguousarray(
            z_loc.reshape(n // P, P, d).transpose(1, 0, 2).reshape(P, -1)
        ).astype(ml_dtypes.bfloat16)                      # [128, (N/128)*D]
        in_maps.append({"ztp": ztp, "zrow": zrow, "ident": ident})
    return in_maps


def gather_loss(results, n=N):
    total = 0.0
    for r in results:
        total += np.asarray(r["out"], dtype=np.float64).sum()
    return np.float32(total / n)


_PROGRAM_CACHE = {}


def kernel(z_i, z_j):
    _ensure_import_path()
    from concourse.bass_utils import run_bass_kernel_spmd

    key = (N, D, RPC)
    if key not in _PROGRAM_CACHE:
        _PROGRAM_CACHE[key] = build_program()
    nc = _PROGRAM_CACHE[key]
    in_maps = make_in_maps(z_i, z_j)
    results = run_bass_kernel_spmd(nc, in_maps, list(range(NCORES))).results
    return gather_loss(results)


if __name__ == "__main__":
    rng = np.random.default_rng(0)
    z_i = rng.standard_normal((B, D), dtype=np.float32)
    z_j = rng.standard_normal((B, D), dtype=np.float32)
    loss = kernel(z_i, z_j)
    print("loss:", loss)


# revision 19
# speedup vs baseline: 1.2204x; 1.1043x over previous
"""SimCLR contrastive loss (NT-Xent) on 8 Trainium2 NeuronCores.

Reference:
    z  = concat(z_i, z_j)                     # [N, D], N = 8192, D = 256
    zn = z / max(||z||_row, eps)
    sim = zn @ zn.T / TEMP                    # TEMP = 0.5
    lse = logsumexp(sim with -inf diagonal, axis=1)
    pos[r] = sim[r, (r + B) mod N]
    loss = sum(lse - pos) / N

Distribution: data-parallel over rows.  Core c owns rows [1024c, 1024c+1024);
the host ships each core the *row-rotated* embeddings so one SPMD program
serves all cores (own rows are always local columns [0, 1024), the positive
window for row-tile m is local columns [4096+128m, 4096+128m+128), and the
diagonal is folded out by subtracting the constant e^2 from each row sum).

Per-core pipeline (v2: fp8 DoubleRow matmul + split exp):
  Host ships two bf16 layouts of the rotated z (pure layout work):
    ztp  [128, 2, 8192]: ztp[p, j, c] = z_loc[c, p + 128j]   (transposed, the
         two 128-deep contraction tiles side by side for DoubleRow packing)
    zrow [128, 16384]:   zrow[p, 64t + ...] = z_loc[128t + p, :]  (row-major,
         for single-pass row-norm computation)
  1. Norms: 64x DVE tensor_tensor_reduce (z*z with accum) -> ss [128, 64]
     in the transposed layout directly; Quake-seed Newton rsqrt on DVE
     (ACT Rsqrt LUT is banned for accuracy); inv -> DRAM (bf16) -> stride-0
     broadcast DMA -> bc [128, 8192]; znb = ztp * bc cast to fp8e4.
  2. Main loop (4 sweeps x 8 row tiles): each [128, 2048] PSUM tile filled by
     4 DoubleRow fp8 matmuls (256-deep contraction, 2 elem/cycle); consumed by
     either ScalarE (exp + fused row-sum accumulate, scale=2 folds 1/TEMP) or
     VectorE (Schraudolph fast-exp: (x*A + B) -> int16 bit pattern == bf16
     exp approximation, then a 4x-mode tensor_scalar with accum_out row-sums
     it; the magic constant zeroes the mean sawtooth bias).
  3. lse = ln(rowsum - e^2); out = lse - pos; host sums in fp64 / N.
"""

import os
import sys

import numpy as np

B = 4096
D = 256
N = 2 * B
NCORES = 8
RPC = N // NCORES  # rows per core

_CANDIDATE_PATHS = ("/opt/trn_rl_repo", "/root/.axon_site/_ro/trn_rl_repo")


def _ensure_import_path():
    try:
        import concourse.bass  # noqa: F401
        return
    except ImportError:
        pass
    for p in _CANDIDATE_PATHS:
        if os.path.isdir(p) and p not in sys.path:
            sys.path.insert(0, p)
    import concourse.bass  # noqa: F401


# Schraudolph fast-exp constants for exp(2*x) via bf16 bit pattern:
#   bits = round(x * 2*128*log2(e) + (127*128 - 128*log2(Eg)))
# where Eg = E_f[(1+f)/2^f] = 1.0406984 zeroes the mean sawtooth bias.
SCH_A = 369.32993046757464          # 2 * 128 * log2(e)
SCH_B = 16248.633                   # 16256 - 128*log2(1.0406984)
EXP2 = float(np.exp(2.0))           # exp(sim[i,i] * 2) subtracted per row

USE_FP8 = True


def build_program(n=N, d=D, rpc=RPC):
    _ensure_import_path()
    from contextlib import ExitStack

    import concourse.bacc as bacc
    import concourse.tile as tile
    from concourse import mybir

    f32 = mybir.dt.float32
    bf16 = mybir.dt.bfloat16
    fp8 = mybir.dt.float8e4
    i16 = mybir.dt.int16
    i32 = mybir.dt.int32
    FT = mybir.ActivationFunctionType
    OP = mybir.AluOpType
    DR = mybir.MatmulPerfMode.DoubleRow

    P = 128
    CH = 512                    # one fp32 PSUM bank
    GW = 2048                   # sweep/group width (4 banks)
    nsw = n // GW               # 4 sweeps
    mt = rpc // P               # 8 row tiles
    tpg = GW // P               # norm t-chunks per group (16)
    EG = GW // CH               # 512-chunks per sweep (4)

    # tiles whose exp+rowsum runs on VectorE (the rest go to ScalarE)
    DVE_TILES = {(1, 3), (1, 6), (2, 2), (2, 5), (3, 1), (3, 3), (3, 5), (3, 7)}

    nc = bacc.Bacc("TRN2", target_bir_lowering=False, debug=False)
    ztp_d = nc.dram_tensor("ztp", [P, 2, n], bf16, kind="ExternalInput").ap()
    zrow_d = nc.dram_tensor("zrow", [P, (n // P) * d], bf16, kind="ExternalInput").ap()
    id_d = nc.dram_tensor("ident", [P, P], f32, kind="ExternalInput").ap()
    out_d = nc.dram_tensor("out", [P, mt], f32, kind="ExternalOutput").ap()
    invd = nc.dram_tensor("invd", [1, n], bf16).ap()   # scratch: 1/norm

    with tile.TileContext(nc) as tc, ExitStack() as ctx:
        big = ctx.enter_context(tc.tile_pool(name="big", bufs=1))
        small = ctx.enter_context(tc.tile_pool(name="small", bufs=2))
        i16p = ctx.enter_context(tc.tile_pool(name="i16p", bufs=2))
        stat = ctx.enter_context(tc.tile_pool(name="stat", bufs=1))
        mps = ctx.enter_context(tc.tile_pool(name="mps", bufs=2, space="PSUM"))

        zt = big.tile([P, 2, n], bf16, tag="zt")
        zrow = big.tile([P, (n // P) * d], bf16, tag="zrow")
        mm_dt = fp8 if USE_FP8 else bf16
        znb = big.tile([P, 2, n], mm_dt, tag="znb")
        zn16 = big.tile([P, 2, n], bf16, tag="zn16")
        jnk = big.tile([P, GW], bf16, tag="jnk")       # DVE exp-sum junk out
        sqb = big.tile([P, tpg * d], bf16, tag="sqb")  # squares scratch
        tjnk = stat.tile([P, d], bf16, tag="tjnk")     # ttr junk out
        ident_sb = stat.tile([P, P], f32, tag="ident")
        sspr = [stat.tile([P, 2 * tpg], f32, tag=f"ss{q}", name=f"ss{q}") for q in range(2)]
        ssg = [sspr[g // 2][:, tpg * (g % 2) : tpg * (g % 2 + 1)] for g in range(nsw)]
        invpr = [stat.tile([P, P], bf16, tag=f"inv{q}", name=f"inv{q}") for q in range(2)]
        invT = [stat.tile([P, P], bf16, tag=f"ivT{q}", name=f"ivT{q}") for q in range(2)]
        irow = [stat.tile([1, GW], bf16, tag=f"ir{g}", name=f"ir{g}") for g in range(nsw)]
        ones1 = stat.tile([1, P], bf16, tag="ones1")
        bcps_t = [None] * nsw
        out_sb = stat.tile([P, mt], f32, tag="out_sb")
        partsA = stat.tile([P, mt, nsw], f32, tag="partsA")
        partsB = stat.tile([P, mt, nsw], f32, tag="partsB")
        poss = stat.tile([P, mt], f32, tag="poss")

        nc.sync.dma_start(out=ident_sb, in_=id_d)
        nc.vector.memset(ones1, 1.0)
        nc.vector.memset(partsA, 0.0)
        nc.vector.memset(partsB, 0.0)

        # ---- input streams: interleave the two layouts group by group so
        # group g's norm inputs and matmul inputs both arrive early; the SP
        # HWDGE FIFO carries only these (no dependent DMAs -> no head-of-line
        # stalls) and drains at HBM line rate.
        for g in range(nsw):
            zr = slice(g * P * tpg * 2, (g + 1) * P * tpg * 2)  # 4096 cols
            nc.sync.dma_start(out=zrow[:, zr], in_=zrow_d[:, zr])
            G = slice(GW * g, GW * (g + 1))
            nc.sync.dma_start(out=zt[:, :, G], in_=ztp_d[:, :, G])

        def norms_group(g):
            """ss for rows [2048g, 2048(g+1)): square (2x bf16 TT) then one
            3D tensor_reduce over the innermost 256-wide row chunks."""
            zc = slice(tpg * g * d, tpg * (g + 1) * d)
            nc.vector.tensor_mul(sqb, zrow[:, zc], zrow[:, zc])
            nc.vector.tensor_reduce(
                out=ssg[g],
                in_=sqb.rearrange("p (t d) -> p t d", d=d),
                axis=mybir.AxisListType.X,
                op=OP.add,
            )

        def norms_group_act(g):
            """Same, on the (idle during prologue) Scalar engine via
            Square with fused accumulate, one 256-chunk at a time."""
            for t in range(tpg):
                zc = slice((tpg * g + t) * d, (tpg * g + t + 1) * d)
                nc.scalar.activation(
                    out=tjnk, in_=zrow[:, zc], func=FT.Square,
                    accum_out=ssg[g][:, t : t + 1],
                )

        def newton_pair(q):
            """inv = 1/sqrt(ss) for group pair q: Quake seed + 2 Newton
            steps, then a 32x32-block DVE transpose so the DRAM write of
            the column-ordered inverse norms is contiguous (a strided
            [128,16]->[1,2048] write costs ~15us in HBM RMW)."""
            sg = sspr[q]
            ii = small.tile([P, 2 * tpg], i32, tag="ii")
            nc.vector.tensor_scalar(
                out=ii, in0=sg.bitcast(i32), scalar1=1, scalar2=None,
                op0=OP.arith_shift_right,
            )
            nc.vector.tensor_scalar(
                out=ii, in0=ii, scalar1=-1, scalar2=None, op0=OP.bitwise_xor
            )
            nc.vector.tensor_scalar(
                out=ii, in0=ii, scalar1=0x5F3759DF + 1, scalar2=None, op0=OP.add
            )
            y = ii.bitcast(f32)
            t_ = small.tile([P, 2 * tpg], f32, tag="t_")
            for _ in range(2):
                nc.vector.tensor_mul(t_, y, y)
                nc.vector.tensor_mul(t_, t_, sg)
                nc.vector.tensor_scalar(
                    out=t_, in0=t_, scalar1=-0.5, scalar2=1.5,
                    op0=OP.mult, op1=OP.add,
                )
                nc.vector.tensor_mul(y, y, t_)
            nc.vector.tensor_copy(out=invpr[q][:, 0 : 2 * tpg], in_=y)
            nc.vector.transpose(out=invT[q], in_=invpr[q])

        def bcast_pair(q):
            """Contiguous [32,128] -> DRAM write of column-ordered inv."""
            Q = slice(2 * GW * q, 2 * GW * (q + 1))
            nc.gpsimd.dma_start(out=invd[0, Q], in_=invT[q][0 : 2 * tpg, :])

        def irow_group(g):
            G = slice(GW * g, GW * (g + 1))
            nc.gpsimd.dma_start(out=irow[g], in_=invd[:, G])


        def bcmm_group(g):
            bcps = mps.tile([P, GW], f32, tag="ps", name=f"bcps{g}")
            for c in range(EG):
                nc.tensor.matmul(
                    bcps[:, CH * c : CH * (c + 1)],
                    ones1,
                    irow[g][0:1, CH * c : CH * (c + 1)],
                    start=True,
                    stop=True,
                )
            bcps_t[g] = bcps

        def normalize_group(g):
            G = slice(GW * g, GW * (g + 1))
            for j in range(2):
                nc.vector.tensor_mul(zn16[:, j, G], zt[:, j, G], bcps_t[g])
            nc.vector.tensor_copy(out=znb[:, :, G], in_=zn16[:, :, G])

        def warmup(src, k, nmm):
            """Dummy matmuls on already-loaded data keep the PE HAM warm
            (any >3.4us idle gap drops the PE clock 2.4 -> 1.2 GHz)."""
            wps = mps.tile([P, GW], f32, tag="ps", name=f"wu{k}")
            for i in range(nmm):
                nc.tensor.matmul(
                    wps[:, 0:CH], src[:, 0:P], src[:, 0:CH],
                    start=True, stop=True,
                )

        # Prologue pipeline: DVE order chosen so group 0's chain finishes
        # fastest while later groups' work fills the DMA-wait gaps; PE
        # warmup matmuls are spaced by DMA arrivals to hold the clock high.
        norms_group(0)
        norms_group_act(2)
        warmup(zrow, 0, 2)
        warmup(zt[:, 0, :], 1, 2)
        norms_group(1)
        newton_pair(0)
        bcast_pair(0)
        irow_group(0)
        irow_group(1)
        warmup(zrow[:, tpg * d :], 2, 2)
        bcmm_group(0)
        normalize_group(0)

        def main_tile(s, m):
            ps = mps.tile([P, GW], f32, tag="ps", name=f"ps_{s}_{m}")
            for c in range(EG):
                cols = slice(GW * s + CH * c, GW * s + CH * (c + 1))
                if USE_FP8:
                    nc.tensor.matmul(
                        ps[:, CH * c : CH * (c + 1)],
                        znb[:, :, P * m : P * (m + 1)],
                        znb[:, :, cols],
                        start=True,
                        stop=True,
                        perf_mode=DR,
                    )
                else:
                    for j in range(2):
                        nc.tensor.matmul(
                            ps[:, CH * c : CH * (c + 1)],
                            znb[:, j, P * m : P * (m + 1)],
                            znb[:, j, cols],
                            start=(j == 0),
                            stop=(j == 1),
                        )
            w0 = n // 2 + P * m
            if w0 // GW == s:  # positive-pair window lives in this sweep
                off = w0 % GW
                junk = small.tile([P, P], f32, tag="pjunk")
                nc.vector.scalar_tensor_tensor(
                    out=junk,
                    in0=ps[:, off : off + P],
                    scalar=2.0,
                    in1=ident_sb,
                    op0=OP.mult,
                    op1=OP.mult,
                    accum_out=poss[:, m : m + 1],
                )
            if (s, m) in DVE_TILES:
                ib = i16p.tile([P, GW], i16, tag="ib", name=f"ib_{s}_{m}")
                nc.vector.tensor_scalar(
                    out=ib, in0=ps, scalar1=SCH_A, scalar2=SCH_B,
                    op0=OP.mult, op1=OP.add,
                )
                nc.vector.tensor_scalar(
                    out=jnk, in0=ib.bitcast(bf16), scalar1=1.0, scalar2=None,
                    op0=OP.mult, op1=OP.add,
                    accum_out=partsB[:, m, s : s + 1],
                )
            else:
                nc.scalar.activation(
                    out=ps,
                    in_=ps,
                    func=FT.Exp,
                    scale=2.0,
                    accum_out=partsA[:, m, s : s + 1],
                )

        tseq = [(s, m) for s in range(nsw) for m in range(mt)]
        for k, (s, m) in enumerate(tseq):
            main_tile(s, m)
            if k == 0:
                norms_group_act(3)
            elif k == 1:
                bcmm_group(1)
                normalize_group(1)
            elif k == 4:
                newton_pair(1)
                bcast_pair(1)
                irow_group(2)
                irow_group(3)
            elif k == 6:
                bcmm_group(2)
                normalize_group(2)
            elif k == 10:
                bcmm_group(3)
                normalize_group(3)

        # ---- Per-row finalization ----
        for m in range(mt):
            SA = small.tile([P, 1], f32, tag="SA")
            nc.vector.tensor_reduce(
                out=SA, in_=partsA[:, m, :], axis=mybir.AxisListType.X, op=OP.add
            )
            SB = small.tile([P, 1], f32, tag="SB")
            nc.vector.tensor_reduce(
                out=SB, in_=partsB[:, m, :], axis=mybir.AxisListType.X, op=OP.add
            )
            nc.vector.tensor_add(SA, SA, SB)
            nc.vector.tensor_scalar_add(SA, SA, -EXP2)
            lse = small.tile([P, 1], f32, tag="lse")
            nc.scalar.activation(out=lse, in_=SA, func=FT.Ln)
            nc.vector.tensor_tensor(
                out=out_sb[:, m : m + 1], in0=lse, in1=poss[:, m : m + 1],
                op=OP.subtract,
            )
        nc.sync.dma_start(out=out_d, in_=out_sb)

    nc.compile()
    return nc


def make_in_maps(z_i, z_j, n=N, d=D, rpc=RPC, ncores=NCORES):
    """Host-side sharding: two rotated bf16 layouts per core (layout only)."""
    import ml_dtypes

    P = 128
    z = np.concatenate(
        [np.asarray(z_i, dtype=np.float32), np.asarray(z_j, dtype=np.float32)],
        axis=0,
    )
    ident = np.eye(P, dtype=np.float32)
    in_maps = []
    for c in range(ncores):
        z_loc = np.roll(z, -rpc * c, axis=0)              # [N, D]
        zT = z_loc.T                                      # [D, N]
        ztp = np.ascontiguousarray(
            zT.reshape(2, P, n).transpose(1, 0, 2)
        ).astype(ml_dtypes.bfloat16)                      # [128, 2, N]
        zrow = np.ascontiguousarray(
            z_loc.reshape(n // P, P, d).transpose(1, 0, 2).reshape(P, -1)
        ).astype(ml_dtypes.bfloat16)                      # [128, (N/128)*D]
        in_maps.append({"ztp": ztp, "zrow": zrow, "ident": ident})
    return in_maps


def gather_loss(results, n=N):
    total = 0.0
    for r in results:
        total += np.asarray(r["out"], dtype=np.float64).sum()
    return np.float32(total / n)


_PROGRAM_CACHE = {}


def kernel(z_i, z_j):
    _ensure_import_path()
    from concourse.bass_utils import run_bass_kernel_spmd

    key = (N, D, RPC)
    if key not in _PROGRAM_CACHE:
        _PROGRAM_CACHE[key] = build_program()
    nc = _PROGRAM_CACHE[key]
    in_maps = make_in_maps(z_i, z_j)
    results = run_bass_kernel_spmd(nc, in_maps, list(range(NCORES))).results
    return gather_loss(results)


if __name__ == "__main__":
    rng = np.random.default_rng(0)
    z_i = rng.standard_normal((B, D), dtype=np.float32)
    z_j = rng.standard_normal((B, D), dtype=np.float32)
    loss = kernel(z_i, z_j)
    print("loss:", loss)


# revision 20
# speedup vs baseline: 1.2585x; 1.0313x over previous
"""SimCLR contrastive loss (NT-Xent) on 8 Trainium2 NeuronCores.

Reference:
    z  = concat(z_i, z_j)                     # [N, D], N = 8192, D = 256
    zn = z / max(||z||_row, eps)
    sim = zn @ zn.T / TEMP                    # TEMP = 0.5
    lse = logsumexp(sim with -inf diagonal, axis=1)
    pos[r] = sim[r, (r + B) mod N]
    loss = sum(lse - pos) / N

Distribution: data-parallel over rows.  Core c owns rows [1024c, 1024c+1024);
the host ships each core the *row-rotated* embeddings so one SPMD program
serves all cores (own rows are always local columns [0, 1024), the positive
window for row-tile m is local columns [4096+128m, 4096+128m+128), and the
diagonal is folded out by subtracting the constant e^2 from each row sum).

Per-core pipeline (v2: fp8 DoubleRow matmul + split exp):
  Host ships two bf16 layouts of the rotated z (pure layout work):
    ztp  [128, 2, 8192]: ztp[p, j, c] = z_loc[c, p + 128j]   (transposed, the
         two 128-deep contraction tiles side by side for DoubleRow packing)
    zrow [128, 16384]:   zrow[p, 64t + ...] = z_loc[128t + p, :]  (row-major,
         for single-pass row-norm computation)
  1. Norms: 64x DVE tensor_tensor_reduce (z*z with accum) -> ss [128, 64]
     in the transposed layout directly; Quake-seed Newton rsqrt on DVE
     (ACT Rsqrt LUT is banned for accuracy); inv -> DRAM (bf16) -> stride-0
     broadcast DMA -> bc [128, 8192]; znb = ztp * bc cast to fp8e4.
  2. Main loop (4 sweeps x 8 row tiles): each [128, 2048] PSUM tile filled by
     4 DoubleRow fp8 matmuls (256-deep contraction, 2 elem/cycle); consumed by
     either ScalarE (exp + fused row-sum accumulate, scale=2 folds 1/TEMP) or
     VectorE (Schraudolph fast-exp: (x*A + B) -> int16 bit pattern == bf16
     exp approximation, then a 4x-mode tensor_scalar with accum_out row-sums
     it; the magic constant zeroes the mean sawtooth bias).
  3. lse = ln(rowsum - e^2); out = lse - pos; host sums in fp64 / N.
"""

import os
import sys

import numpy as np

B = 4096
D = 256
N = 2 * B
NCORES = 8
RPC = N // NCORES  # rows per core

_CANDIDATE_PATHS = ("/opt/trn_rl_repo", "/root/.axon_site/_ro/trn_rl_repo")


def _ensure_import_path():
    try:
        import concourse.bass  # noqa: F401
        return
    except ImportError:
        pass
    for p in _CANDIDATE_PATHS:
        if os.path.isdir(p) and p not in sys.path:
            sys.path.insert(0, p)
    import concourse.bass  # noqa: F401


# Schraudolph fast-exp constants for exp(2*x) via bf16 bit pattern:
#   bits = round(x * 2*128*log2(e) + (127*128 - 128*log2(Eg)))
# where Eg = E_f[(1+f)/2^f] = 1.0406984 zeroes the mean sawtooth bias.
SCH_A = 369.32993046757464          # 2 * 128 * log2(e)
SCH_B = 16248.633                   # 16256 - 128*log2(1.0406984)
EXP2 = float(np.exp(2.0))           # exp(sim[i,i] * 2) subtracted per row

USE_FP8 = True


def build_program(n=N, d=D, rpc=RPC):
    _ensure_import_path()
    from contextlib import ExitStack

    import concourse.bacc as bacc
    import concourse.tile as tile
    from concourse import mybir

    f32 = mybir.dt.float32
    bf16 = mybir.dt.bfloat16
    fp8 = mybir.dt.float8e4
    i16 = mybir.dt.int16
    i32 = mybir.dt.int32
    FT = mybir.ActivationFunctionType
    OP = mybir.AluOpType
    DR = mybir.MatmulPerfMode.DoubleRow

    P = 128
    CH = 512                    # one fp32 PSUM bank
    GW = 2048                   # sweep/group width (4 banks)
    nsw = n // GW               # 4 sweeps
    mt = rpc // P               # 8 row tiles
    tpg = GW // P               # norm t-chunks per group (16)
    EG = GW // CH               # 512-chunks per sweep (4)

    # tiles whose exp+rowsum runs on VectorE (the rest go to ScalarE)
    DVE_TILES = {(1, 3), (1, 6), (2, 2), (2, 5), (3, 1), (3, 3), (3, 5), (3, 7)}

    nc = bacc.Bacc("TRN2", target_bir_lowering=False, debug=False)
    ztp_d = nc.dram_tensor("ztp", [P, 2, n], bf16, kind="ExternalInput").ap()
    zrow_d = nc.dram_tensor("zrow", [P, (n // P) * d], bf16, kind="ExternalInput").ap()
    id_d = nc.dram_tensor("ident", [P, P], f32, kind="ExternalInput").ap()
    idb_d = nc.dram_tensor("identb", [P, P], bf16, kind="ExternalInput").ap()
    out_d = nc.dram_tensor("out", [P, mt], f32, kind="ExternalOutput").ap()
    invd = nc.dram_tensor("invd", [1, n], bf16).ap()   # scratch: 1/norm

    with tile.TileContext(nc) as tc, ExitStack() as ctx:
        big = ctx.enter_context(tc.tile_pool(name="big", bufs=1))
        small = ctx.enter_context(tc.tile_pool(name="small", bufs=2))
        i16p = ctx.enter_context(tc.tile_pool(name="i16p", bufs=2))
        stat = ctx.enter_context(tc.tile_pool(name="stat", bufs=1))
        mps = ctx.enter_context(tc.tile_pool(name="mps", bufs=2, space="PSUM"))

        zt = big.tile([P, 2, n], bf16, tag="zt")
        zrow = big.tile([P, (n // P) * d], bf16, tag="zrow")
        mm_dt = fp8 if USE_FP8 else bf16
        znb = big.tile([P, 2, n], mm_dt, tag="znb")
        zn16 = big.tile([P, 2, n], bf16, tag="zn16")
        jnk = big.tile([P, GW], bf16, tag="jnk")       # DVE exp-sum junk out
        sqb = big.tile([P, tpg * d], bf16, tag="sqb")  # squares scratch
        tjnk = stat.tile([P, d], bf16, tag="tjnk")     # ttr junk out
        ident_sb = stat.tile([P, P], f32, tag="ident")
        identb_sb = stat.tile([P, P], bf16, tag="identb")
        sspr = [stat.tile([P, 2 * tpg], f32, tag=f"ss{q}", name=f"ss{q}") for q in range(2)]
        ssg = [sspr[g // 2][:, tpg * (g % 2) : tpg * (g % 2 + 1)] for g in range(nsw)]
        invpr = [stat.tile([P, P], bf16, tag=f"inv{q}", name=f"inv{q}") for q in range(2)]
        invT = [stat.tile([2 * tpg, P], bf16, tag=f"ivT{q}", name=f"ivT{q}") for q in range(2)]
        irow = [stat.tile([1, GW], bf16, tag=f"ir{g}", name=f"ir{g}") for g in range(nsw)]
        ones1 = stat.tile([1, P], bf16, tag="ones1")
        bcps_t = [None] * nsw
        out_sb = stat.tile([P, mt], f32, tag="out_sb")
        partsA = stat.tile([P, mt, nsw], f32, tag="partsA")
        partsB = stat.tile([P, mt, nsw], f32, tag="partsB")
        poss = stat.tile([P, mt], f32, tag="poss")

        nc.sync.dma_start(out=ident_sb, in_=id_d)
        nc.sync.dma_start(out=identb_sb, in_=idb_d)
        nc.vector.memset(ones1, 1.0)
        nc.vector.memset(partsA, 0.0)
        nc.vector.memset(partsB, 0.0)

        # ---- input streams: interleave the two layouts group by group so
        # group g's norm inputs and matmul inputs both arrive early; the SP
        # HWDGE FIFO carries only these (no dependent DMAs -> no head-of-line
        # stalls) and drains at HBM line rate.
        for g in range(nsw):
            zr = slice(g * P * tpg * 2, (g + 1) * P * tpg * 2)  # 4096 cols
            nc.sync.dma_start(out=zrow[:, zr], in_=zrow_d[:, zr])
            G = slice(GW * g, GW * (g + 1))
            nc.sync.dma_start(out=zt[:, :, G], in_=ztp_d[:, :, G])

        def norms_group(g):
            """ss for rows [2048g, 2048(g+1)): square (2x bf16 TT) then one
            3D tensor_reduce over the innermost 256-wide row chunks."""
            zc = slice(tpg * g * d, tpg * (g + 1) * d)
            nc.vector.tensor_mul(sqb, zrow[:, zc], zrow[:, zc])
            nc.vector.tensor_reduce(
                out=ssg[g],
                in_=sqb.rearrange("p (t d) -> p t d", d=d),
                axis=mybir.AxisListType.X,
                op=OP.add,
            )

        def norms_group_act(g):
            """Same, on the (idle during prologue) Scalar engine via
            Square with fused accumulate, one 256-chunk at a time."""
            for t in range(tpg):
                zc = slice((tpg * g + t) * d, (tpg * g + t + 1) * d)
                nc.scalar.activation(
                    out=tjnk, in_=zrow[:, zc], func=FT.Square,
                    accum_out=ssg[g][:, t : t + 1],
                )

        def newton_pair(q):
            """inv = 1/sqrt(ss) for group pair q: Quake seed + 2 Newton
            steps, then a 32x32-block DVE transpose so the DRAM write of
            the column-ordered inverse norms is contiguous (a strided
            [128,16]->[1,2048] write costs ~15us in HBM RMW)."""
            sg = sspr[q]
            ii = small.tile([P, 2 * tpg], i32, tag="ii")
            nc.vector.tensor_scalar(
                out=ii, in0=sg.bitcast(i32), scalar1=1, scalar2=None,
                op0=OP.arith_shift_right,
            )
            nc.vector.tensor_scalar(
                out=ii, in0=ii, scalar1=-1, scalar2=None, op0=OP.bitwise_xor
            )
            nc.vector.tensor_scalar(
                out=ii, in0=ii, scalar1=0x5F3759DF + 1, scalar2=None, op0=OP.add
            )
            y = ii.bitcast(f32)
            t_ = small.tile([P, 2 * tpg], f32, tag="t_")
            for _ in range(2):
                nc.vector.tensor_mul(t_, y, y)
                nc.vector.tensor_mul(t_, t_, sg)
                nc.vector.tensor_scalar(
                    out=t_, in0=t_, scalar1=-0.5, scalar2=1.5,
                    op0=OP.mult, op1=OP.add,
                )
                nc.vector.tensor_mul(y, y, t_)
            nc.vector.tensor_copy(out=invpr[q][:, 0 : 2 * tpg], in_=y)
            tps = mps.tile([P, GW], f32, tag="ps", name=f"tp{q}")
            tview = tps[:, 0 : P // 2].bitcast(bf16)
            nc.tensor.transpose(tview, invpr[q], identb_sb)
            nc.vector.tensor_copy(out=invT[q], in_=tview[0 : 2 * tpg, :])

        def bcast_pair(q):
            """Contiguous [32,128] -> DRAM write of column-ordered inv."""
            Q = slice(2 * GW * q, 2 * GW * (q + 1))
            nc.gpsimd.dma_start(out=invd[0, Q], in_=invT[q])

        def irow_group(g):
            G = slice(GW * g, GW * (g + 1))
            nc.gpsimd.dma_start(out=irow[g], in_=invd[:, G])


        def bcmm_group(g):
            bcps = mps.tile([P, GW], f32, tag="ps", name=f"bcps{g}")
            for c in range(EG):
                nc.tensor.matmul(
                    bcps[:, CH * c : CH * (c + 1)],
                    ones1,
                    irow[g][0:1, CH * c : CH * (c + 1)],
                    start=True,
                    stop=True,
                )
            bcps_t[g] = bcps

        def normalize_group(g):
            G = slice(GW * g, GW * (g + 1))
            for j in range(2):
                nc.vector.tensor_mul(zn16[:, j, G], zt[:, j, G], bcps_t[g])
            nc.vector.tensor_copy(out=znb[:, :, G], in_=zn16[:, :, G])

        def warmup(src, k, nmm):
            """Dummy matmuls on already-loaded data keep the PE HAM warm
            (any >3.4us idle gap drops the PE clock 2.4 -> 1.2 GHz)."""
            wps = mps.tile([P, GW], f32, tag="ps", name=f"wu{k}")
            for i in range(nmm):
                nc.tensor.matmul(
                    wps[:, 0:CH], src[:, 0:P], src[:, 0:CH],
                    start=True, stop=True,
                )

        # Prologue pipeline: DVE order chosen so group 0's chain finishes
        # fastest while later groups' work fills the DMA-wait gaps; PE
        # warmup matmuls are spaced by DMA arrivals to hold the clock high.
        norms_group(0)
        norms_group_act(2)
        warmup(zrow, 0, 2)
        warmup(zt[:, 0, :], 1, 2)
        norms_group(1)
        newton_pair(0)
        bcast_pair(0)
        irow_group(0)
        irow_group(1)
        warmup(zrow[:, tpg * d :], 2, 2)
        bcmm_group(0)
        normalize_group(0)

        def main_tile(s, m):
            ps = mps.tile([P, GW], f32, tag="ps", name=f"ps_{s}_{m}")
            for c in range(EG):
                cols = slice(GW * s + CH * c, GW * s + CH * (c + 1))
                if USE_FP8:
                    nc.tensor.matmul(
                        ps[:, CH * c : CH * (c + 1)],
                        znb[:, :, P * m : P * (m + 1)],
                        znb[:, :, cols],
                        start=True,
                        stop=True,
                        perf_mode=DR,
                    )
                else:
                    for j in range(2):
                        nc.tensor.matmul(
                            ps[:, CH * c : CH * (c + 1)],
                            znb[:, j, P * m : P * (m + 1)],
                            znb[:, j, cols],
                            start=(j == 0),
                            stop=(j == 1),
                        )
            w0 = n // 2 + P * m
            if w0 // GW == s:  # positive-pair window lives in this sweep
                off = w0 % GW
                junk = small.tile([P, P], f32, tag="pjunk")
                nc.vector.scalar_tensor_tensor(
                    out=junk,
                    in0=ps[:, off : off + P],
                    scalar=2.0,
                    in1=ident_sb,
                    op0=OP.mult,
                    op1=OP.mult,
                    accum_out=poss[:, m : m + 1],
                )
            if (s, m) in DVE_TILES:
                ib = i16p.tile([P, GW], i16, tag="ib", name=f"ib_{s}_{m}")
                nc.vector.tensor_scalar(
                    out=ib, in0=ps, scalar1=SCH_A, scalar2=SCH_B,
                    op0=OP.mult, op1=OP.add,
                )
                nc.vector.tensor_scalar(
                    out=jnk, in0=ib.bitcast(bf16), scalar1=1.0, scalar2=None,
                    op0=OP.mult, op1=OP.add,
                    accum_out=partsB[:, m, s : s + 1],
                )
            else:
                nc.scalar.activation(
                    out=ps,
                    in_=ps,
                    func=FT.Exp,
                    scale=2.0,
                    accum_out=partsA[:, m, s : s + 1],
                )

        tseq = [(s, m) for s in range(nsw) for m in range(mt)]
        for k, (s, m) in enumerate(tseq):
            main_tile(s, m)
            if k == 0:
                norms_group_act(3)
            elif k == 1:
                bcmm_group(1)
                normalize_group(1)
            elif k == 4:
                newton_pair(1)
                bcast_pair(1)
                irow_group(2)
                irow_group(3)
            elif k == 6:
                bcmm_group(2)
                normalize_group(2)
            elif k == 10:
                bcmm_group(3)
                normalize_group(3)

        # ---- Per-row finalization ----
        for m in range(mt):
            SA = small.tile([P, 1], f32, tag="SA")
            nc.vector.tensor_reduce(
                out=SA, in_=partsA[:, m, :], axis=mybir.AxisListType.X, op=OP.add
            )
            SB = small.tile([P, 1], f32, tag="SB")
            nc.vector.tensor_reduce(
                out=SB, in_=partsB[:, m, :], axis=mybir.AxisListType.X, op=OP.add
            )
            nc.vector.tensor_add(SA, SA, SB)
            nc.vector.tensor_scalar_add(SA, SA, -EXP2)
            lse = small.tile([P, 1], f32, tag="lse")
            nc.scalar.activation(out=lse, in_=SA, func=FT.Ln)
            nc.vector.tensor_tensor(
                out=out_sb[:, m : m + 1], in0=lse, in1=poss[:, m : m + 1],
                op=OP.subtract,
            )
        nc.sync.dma_start(out=out_d, in_=out_sb)

    nc.compile()
    return nc


def make_in_maps(z_i, z_j, n=N, d=D, rpc=RPC, ncores=NCORES):
    """Host-side sharding: two rotated bf16 layouts per core (layout only)."""
    import ml_dtypes

    P = 128
    z = np.concatenate(
        [np.asarray(z_i, dtype=np.float32), np.asarray(z_j, dtype=np.float32)],
        axis=0,
    )
    ident = np.eye(P, dtype=np.float32)
    in_maps = []
    for c in range(ncores):
        z_loc = np.roll(z, -rpc * c, axis=0)              # [N, D]
        zT = z_loc.T                                      # [D, N]
        ztp = np.ascontiguousarray(
            zT.reshape(2, P, n).transpose(1, 0, 2)
        ).astype(ml_dtypes.bfloat16)                      # [128, 2, N]
        zrow = np.ascontiguousarray(
            z_loc.reshape(n // P, P, d).transpose(1, 0, 2).reshape(P, -1)
        ).astype(ml_dtypes.bfloat16)                      # [128, (N/128)*D]
        in_maps.append({"ztp": ztp, "zrow": zrow, "ident": ident,
                        "identb": ident.astype(ml_dtypes.bfloat16)})
    return in_maps


def gather_loss(results, n=N):
    total = 0.0
    for r in results:
        total += np.asarray(r["out"], dtype=np.float64).sum()
    return np.float32(total / n)


_PROGRAM_CACHE = {}


def kernel(z_i, z_j):
    _ensure_import_path()
    from concourse.bass_utils import run_bass_kernel_spmd

    key = (N, D, RPC)
    if key not in _PROGRAM_CACHE:
        _PROGRAM_CACHE[key] = build_program()
    nc = _PROGRAM_CACHE[key]
    in_maps = make_in_maps(z_i, z_j)
    results = run_bass_kernel_spmd(nc, in_maps, list(range(NCORES))).results
    return gather_loss(results)


if __name__ == "__main__":
    rng = np.random.default_rng(0)
    z_i = rng.standard_normal((B, D), dtype=np.float32)
    z_j = rng.standard_normal((B, D), dtype=np.float32)
    loss = kernel(z_i, z_j)
    print("loss:", loss)
